# revision 3
# baseline (speedup 1.0000x reference)
"""AFNB (asymmetric fusion non-local block) Trainium2 kernel, 8-core SPMD.

Data-parallel over batch: 16 batches -> 2 per core, no collectives.

Algebra (per batch, softmax over the QUERY axis allows folding):
  theta = w_theta @ y        [IC, N]   (split2: bf16 weights, y = yh+yl bf16)
  th_spp = SPP(theta)        [IC, S]   (exact fp32 max-pool)
  g    = w_g @ y             [IC, N]   (bf16)
  g_spp = SPP(g)             [IC, S]   (bf16)
  M1T  = w_phi^T @ th_spp    [C, S]    (fp32)
  scoresT = M1T^T @ x        [S, N]    (split2: bf16 M1, x = xh+xl)
  e    = exp(scoresT - rowmax)         (ACT, accum -> denom)
  M2T  = (g_spp^T @ w_mask^T) / denom  [S, C] (bf16)
  out  = M2T^T @ e + x       [C, N]
"""

import numpy as np
import ml_dtypes

import concourse.bass as bass
import concourse.tile as tile
from concourse import bacc, mybir
from concourse.ap import AP
from concourse.bass_utils import run_bass_kernel_spmd

F32 = mybir.dt.float32
BF16 = mybir.dt.bfloat16
AX = mybir.AxisListType
OP = mybir.AluOpType

B, C, HH, WW = 16, 512, 64, 64
N = HH * WW
IC = 256
NCORES = 8
NB = B // NCORES  # batches per core
P = 128
KC = C // P   # 4 contraction chunks over C
MI = IC // P  # 2 chunks over IC
NN = 8        # n-chunks
NT = N // NN  # 512
OUT_SIZES = [1, 3, 6, 8]
S = sum(o * o for o in OUT_SIZES)  # 110
SPP_OFF = {1: 0, 3: 1, 6: 10, 8: 46}


def _bounds(n, o):
    return [((i * n) // o, ((i + 1) * n + o - 1) // o) for i in range(o)]


def _atoms():
    bs = set()
    for o in OUT_SIZES:
        for s, e in _bounds(HH, o):
            bs.add(s); bs.add(e)
    bs = sorted(bs)
    return [(bs[i], bs[i + 1]) for i in range(len(bs) - 1)]


ATOMS = _atoms()          # 16 atomic row intervals
NA = len(ATOMS)


def _bin_atom_ranges(o):
    """For each bin of size-o pooling: (first_atom_idx, last_atom_idx_excl)."""
    out = []
    for s, e in _bounds(HH, o):
        a0 = next(i for i, (as_, _) in enumerate(ATOMS) if as_ == s)
        a1 = next(i for i, (_, ae) in enumerate(ATOMS) if ae == e) + 1
        out.append((a0, a1))
    return out


def _grouped(items):
    """Group indices j of (start, length) items into classes {j = r mod m} where
    each class has constant length and arithmetic starts. Returns list of
    (j0, m, cnt, start0, dstart, length)."""
    n = len(items)
    for m in range(1, n + 1):
        groups = []
        ok = True
        for r in range(m):
            js = list(range(r, n, m))
            lens = {items[j][1] for j in js}
            if len(lens) != 1:
                ok = False; break
            starts = [items[j][0] for j in js]
            d = starts[1] - starts[0] if len(starts) > 1 else 0
            if any(starts[i + 1] - starts[i] != d for i in range(len(starts) - 1)):
                ok = False; break
            groups.append((r, m, len(js), starts[0], d, lens.pop()))
        if ok:
            return groups
    raise AssertionError


ROW_GROUPS = {o: _grouped([(a0, a1 - a0) for a0, a1 in _bin_atom_ranges(o)])
              for o in OUT_SIZES}
COL_GROUPS = {o: _grouped([(s, e - s) for s, e in _bounds(WW, o)])
              for o in OUT_SIZES}
RB_OFF = {}  # row-bin output offset (units of 64 cols) per o
_off = 0
for _o in OUT_SIZES:
    RB_OFF[_o] = _off
    _off += _o
RB_TOT = _off  # 18


def mk(ap_base, off_elems, dims):
    """Custom free-dim AP over a tile's base AP: dims = [(step, count), ...]."""
    part = list(ap_base.ap[0])
    return AP(tensor=ap_base.tensor, offset=ap_base.offset + off_elems,
              ap=[part] + [[s, c] for s, c in dims])


def build():
    nc = bacc.Bacc("TRN2", target_bir_lowering=False, debug=False,
                   num_devices=NCORES)
    x_ext = nc.declare_dram_parameter("x", [NB, C, N], F32, isOutput=False)
    y_ext = nc.declare_dram_parameter("y", [NB, C, N], F32, isOutput=False)
    wth_ext = nc.declare_dram_parameter("wthT", [C, IC], BF16, isOutput=False)
    wg_ext = nc.declare_dram_parameter("wgT", [C, IC], BF16, isOutput=False)
    wphi_ext = nc.declare_dram_parameter("wphi", [IC, C], F32, isOutput=False)
    wmk_ext = nc.declare_dram_parameter("wmkT", [IC, C], BF16, isOutput=False)
    out_ext = nc.declare_dram_parameter("out", [NB, C, N], F32, isOutput=True)

    with tile.TileContext(nc) as tc:
        with (
            tc.tile_pool(name="w", bufs=1) as wp,
            tc.tile_pool(name="io32", bufs=6) as iop,
            tc.tile_pool(name="hlc", bufs=10) as hlp,
            tc.tile_pool(name="pool", bufs=1) as pp,
            tc.tile_pool(name="attn", bufs=2) as ap_,
            tc.tile_pool(name="ostg", bufs=6) as osp,
            tc.tile_pool(name="psum", bufs=1, space="PSUM") as ps,
        ):
            # ---- weights (resident) ----
            wth_t = [wp.tile([P, IC], BF16, tag=f"wth{k}", name=f"wth{k}") for k in range(KC)]
            wg_t = [wp.tile([P, IC], BF16, tag=f"wg{k}", name=f"wg{k}") for k in range(KC)]
            wphi_t = [wp.tile([P, C], F32, tag=f"wphi{k}", name=f"wphi{k}") for k in range(MI)]
            wmk_t = [wp.tile([P, C], BF16, tag=f"wmk{k}", name=f"wmk{k}") for k in range(MI)]
            for k in range(KC):
                nc.sync.dma_start(wth_t[k][:], wth_ext[k * P:(k + 1) * P, :])
                nc.sync.dma_start(wg_t[k][:], wg_ext[k * P:(k + 1) * P, :])
            for k in range(MI):
                nc.sync.dma_start(wphi_t[k][:], wphi_ext[k * P:(k + 1) * P, :])
                nc.sync.dma_start(wmk_t[k][:], wmk_ext[k * P:(k + 1) * P, :])

            for b in range(NB):
                emit_batch(nc, tc, b, x_ext, y_ext, out_ext,
                           wth_t, wg_t, wphi_t, wmk_t, iop, hlp, pp, ap_, osp, ps)

    nc.compile()
    return nc


def spp_reduce_from_psum(nc, pt, nn, ratoms, mi, dtag):
    """Stage R: row-atom max-pool directly from a conv psum chunk [128,512]
    (8 image rows). Writes ratom[:, a*WW : (a+1)*WW]."""
    base = pt[:]
    for ai, (s, e) in enumerate(ATOMS):
        if s >= 8 * nn and e <= 8 * (nn + 1):
            ls = s - 8 * nn
            src = mk(base, ls * WW, [(1, WW), (WW, e - s)])
            dst = mk(ratoms[mi][:], ai * WW, [(1, WW)])
            nc.vector.reduce_max(dst, src, axis=AX.X)


def spp_bins(nc, ratoms, rbs, spps):
    """Stages B1 (row bins from row atoms) + B2 (col bins, strided groups)."""
    for mi in range(MI):
        ratom, rb, spp = ratoms[mi][:], rbs[mi][:], spps[mi][:]
        for o in OUT_SIZES:
            for (r, m, cnt, a0, da, ln) in ROW_GROUPS[o]:
                # in: [P, (i cnt, stride da*WW), (w WW, 1), (atoms ln, WW)]
                src = mk(ratom, a0 * WW, [(da * WW, cnt), (1, WW), (WW, ln)])
                dst = mk(rb, (RB_OFF[o] + r) * WW, [(m * WW, cnt), (1, WW)])
                nc.vector.reduce_max(dst, src, axis=AX.X)
            for (r, m, cnt, s0, ds, ln) in COL_GROUPS[o]:
                # in: [P, (i o, stride WW), (j cnt, stride ds), (w ln, 1)]
                src = mk(rb, RB_OFF[o] * WW + s0, [(WW, o), (ds, cnt), (1, ln)])
                dst = mk(spp, SPP_OFF[o] + r, [(o, o), (m, cnt)])
                nc.vector.reduce_max(dst, src, axis=AX.X)


def emit_batch(nc, tc, b, x_ext, y_ext, out_ext, wth_t, wg_t, wphi_t, wmk_t,
               iop, hlp, pp, ap_, osp, ps):
    # ---- load y ----
    y_t = []
    for k in range(KC):
        t = iop.tile([P, N], F32, tag="io32", name=f"yt_{b}_{k}")
        nc.sync.dma_start(t[:], y_ext[b, k * P:(k + 1) * P, :])
        y_t.append(t)
    # x loaded now too (DMA engines have slack; io32 pool slots gate it)
    x_t = []
    for k in range(KC):
        t = iop.tile([P, N], F32, tag="io32", name=f"xt_{b}_{k}")
        nc.sync.dma_start(t[:], x_ext[b, k * P:(k + 1) * P, :])
        x_t.append(t)

    ratoms_th = [pp.tile([P, NA * WW], F32, tag=f"rath{mi}", name=f"rath{mi}_{b}") for mi in range(MI)]
    ratoms_g = [pp.tile([P, NA * WW], BF16, tag=f"rag{mi}", name=f"rag{mi}_{b}") for mi in range(MI)]

    # ---- conv phase: theta (split2) + g (bf16), pooling fused from psum ----
    for nn in range(NN):
        yh_c, yl_c = [], []
        for k in range(KC):
            ysl = y_t[k][:, nn * NT:(nn + 1) * NT]
            h = hlp.tile([P, NT], BF16, tag="hlc", name=f"h_{b}_{nn}_{k}")
            nc.scalar.copy(h[:], ysl)                      # ACT: hi cast
            l = hlp.tile([P, NT], BF16, tag="hlc", name=f"l_{b}_{nn}_{k}")
            eng = nc.vector if k % 2 == 0 else nc.gpsimd
            eng.tensor_sub(l[:], ysl, h[:])                # lo residual
            yh_c.append(h); yl_c.append(l)
        for mi in range(MI):
            pt = ps.tile([P, NT], F32, tag=f"pth{mi}", name=f"pth{mi}_{b}_{nn}")
            for k in range(KC):
                nc.tensor.matmul(pt[:], wth_t[k][:, mi * P:(mi + 1) * P],
                                 yh_c[k][:], start=(k == 0), stop=False)
            for k in range(KC):
                nc.tensor.matmul(pt[:], wth_t[k][:, mi * P:(mi + 1) * P],
                                 yl_c[k][:], start=False, stop=(k == KC - 1))
            spp_reduce_from_psum(nc, pt, nn, ratoms_th, mi, "th")
        for mi in range(MI):
            pg = ps.tile([P, NT], F32, tag=f"pg{mi}", name=f"pg{mi}_{b}_{nn}")
            for k in range(KC):
                nc.tensor.matmul(pg[:], wg_t[k][:, mi * P:(mi + 1) * P],
                                 yh_c[k][:], start=(k == 0), stop=(k == KC - 1))
            spp_reduce_from_psum(nc, pg, nn, ratoms_g, mi, "g")

    # ---- SPP bins ----
    rbs_th = [pp.tile([P, RB_TOT * WW], F32, tag=f"rbth{mi}", name=f"rbth{mi}_{b}") for mi in range(MI)]
    rbs_g = [pp.tile([P, RB_TOT * WW], BF16, tag=f"rbg{mi}", name=f"rbg{mi}_{b}") for mi in range(MI)]
    spp_th = [pp.tile([P, S], F32, tag=f"spth{mi}{b % 2}", name=f"spth{mi}_{b}") for mi in range(MI)]
    spp_g = [pp.tile([P, S], BF16, tag=f"spg{mi}{b % 2}", name=f"spg{mi}_{b}") for mi in range(MI)]
    spp_bins(nc, ratoms_th, rbs_th, spp_th)
    spp_bins(nc, ratoms_g, rbs_g, spp_g)

    # ---- M1T = w_phi^T @ th_spp (fp32), evac to bf16 lhsT chunks ----
    m1_bf = []
    for mc in range(KC):
        pm = ps.tile([P, S], F32, tag="psmall", name=f"pm1_{b}_{mc}")
        for k in range(MI):
            nc.tensor.matmul(pm[:], wphi_t[k][:, mc * P:(mc + 1) * P],
                             spp_th[k][:], start=(k == 0), stop=(k == MI - 1))
        m = pp.tile([P, S], BF16, tag=f"m1_{mc}{b % 2}", name=f"m1b_{b}_{mc}")
        nc.scalar.copy(m[:], pm[:])
        m1_bf.append(m)

    # ---- scores^T = M1^T @ (xh+xl), softmax stats ----
    sc_sb = pp.tile([S, N], F32, tag="scsb")
    gm = pp.tile([S, NN + 8], F32, tag=f"gm{b % 2}")
    for nn in range(NN):
        xh_c, xl_c = [], []
        for k in range(KC):
            xsl = x_t[k][:, nn * NT:(nn + 1) * NT]
            h = hlp.tile([P, NT], BF16, tag="hlc", name=f"h_{b}_{nn}_{k}")
            nc.scalar.copy(h[:], xsl)
            l = hlp.tile([P, NT], BF16, tag="hlc", name=f"l_{b}_{nn}_{k}")
            eng = nc.vector if k % 2 == 0 else nc.gpsimd
            eng.tensor_sub(l[:], xsl, h[:])
            xh_c.append(h); xl_c.append(l)
        psc = ps.tile([S, NT], F32, tag="pattn", name=f"psc_{b}_{nn}")
        for k in range(KC):
            nc.tensor.matmul(psc[:], m1_bf[k][:], xh_c[k][:],
                             start=(k == 0), stop=False)
        for k in range(KC):
            nc.tensor.matmul(psc[:], m1_bf[k][:], xl_c[k][:],
                             start=False, stop=(k == KC - 1))
        nc.vector.reduce_max(gm[:, nn:nn + 1], psc[:], axis=AX.X)
        nc.scalar.copy(sc_sb[:, nn * NT:(nn + 1) * NT], psc[:])

    gmax = pp.tile([S, 1], F32, tag=f"gmax{b % 2}")
    nc.vector.reduce_max(gmax[:], gm[:, 0:NN], axis=AX.X)
    ngmax = pp.tile([S, 1], F32, tag=f"ngmax{b % 2}")
    nc.vector.tensor_scalar_mul(ngmax[:], gmax[:], -1.0)

    e_bf = pp.tile([S, N], BF16, tag="ebf")
    dsum = pp.tile([S, 1], F32, tag=f"dsum{b % 2}")
    nc.scalar.activation(e_bf[:], sc_sb[:], mybir.ActivationFunctionType.Exp,
                         bias=ngmax[:], scale=1.0, accum_out=dsum[:])
    rden = pp.tile([S, 1], F32, tag=f"rden{b % 2}")
    nc.vector.reciprocal(rden[:], dsum[:])

    # ---- M2T = (g_spp^T @ w_mask^T) * rden, bf16 ----
    pm2 = ps.tile([S, C], F32, tag="psmall2", name=f"pm2_{b}")
    for k in range(MI):
        nc.tensor.matmul(pm2[:], spp_g[k][:], wmk_t[k][:],
                         start=(k == 0), stop=(k == MI - 1))
    m2_bf = pp.tile([S, C], BF16, tag=f"m2{b % 2}")
    nc.vector.tensor_scalar_mul(m2_bf[:], pm2[:], rden[:])

    # ---- mask = M2^T @ e ; out = mask + x ----
    for mc in range(KC):
        for nn in range(NN):
            pk = ps.tile([P, NT], F32, tag="pattn", name=f"pk_{b}_{mc}_{nn}")
            nc.tensor.matmul(pk[:], m2_bf[:, mc * P:(mc + 1) * P],
                             e_bf[:, nn * NT:(nn + 1) * NT],
                             start=True, stop=True)
            o = osp.tile([P, NT], F32, tag="ostg", name=f"ost_{b}_{mc}_{nn}")
            nc.scalar.copy(o[:], pk[:])
            nc.gpsimd.tensor_add(o[:], o[:], x_t[mc][:, nn * NT:(nn + 1) * NT])
            nc.sync.dma_start(
                out_ext[b, mc * P:(mc + 1) * P, nn * NT:(nn + 1) * NT], o[:])


_NC_CACHE = {}


def _get_nc():
    if "nc" not in _NC_CACHE:
        _NC_CACHE["nc"] = build()
    return _NC_CACHE["nc"]


def kernel(x, y, w_phi, w_theta, w_g, w_mask):
    x = np.ascontiguousarray(np.asarray(x, dtype=np.float32))
    y = np.ascontiguousarray(np.asarray(y, dtype=np.float32))
    bf = ml_dtypes.bfloat16
    wthT = np.ascontiguousarray(np.asarray(w_theta, np.float32).T).astype(bf)
    wgT = np.ascontiguousarray(np.asarray(w_g, np.float32).T).astype(bf)
    wphi = np.ascontiguousarray(np.asarray(w_phi, np.float32))
    wmkT = np.ascontiguousarray(np.asarray(w_mask, np.float32).T).astype(bf)

    nc = _get_nc()
    in_maps = []
    for c in range(NCORES):
        sl = slice(c * NB, (c + 1) * NB)
        in_maps.append({
            "x": x[sl].reshape(NB, C, N),
            "y": y[sl].reshape(NB, C, N),
            "wthT": wthT, "wgT": wgT, "wphi": wphi, "wmkT": wmkT,
        })
    res = run_bass_kernel_spmd(nc, in_maps, core_ids=list(range(NCORES)))
    out = np.concatenate([r["out"].reshape(NB, C, HH, WW) for r in res.results],
                         axis=0)
    return out


# revision 4
# speedup vs baseline: 1.3863x; 1.3863x over previous
"""AFNB (asymmetric fusion non-local block) Trainium2 kernel, 8-core SPMD.

Data-parallel over batch: 16 batches -> 2 per core, no collectives.

Algebra (per batch, softmax over the QUERY axis allows folding):
  theta = w_theta @ y        [IC, N]   (split2: bf16 weights, y = yh+yl bf16)
  th_spp = SPP(theta)        [IC, S]   (exact fp32 max-pool)
  g    = w_g @ y             [IC, N]   (bf16)
  g_spp = SPP(g)             [IC, S]   (bf16)
  M1T  = w_phi^T @ th_spp    [C, S]    (fp32)
  scoresT = M1T^T @ x        [S, N]    (split2: bf16 M1, x = xh+xl)
  e    = exp(scoresT - rowmax)         (ACT, accum -> denom)
  M2T  = (g_spp^T @ w_mask^T) / denom  [S, C] (bf16)
  out  = M2T^T @ e + x       [C, N]
"""

import numpy as np
import ml_dtypes

import concourse.bass as bass
import concourse.tile as tile
from concourse import bacc, mybir
from concourse.ap import AP
from concourse.bass_utils import run_bass_kernel_spmd

F32 = mybir.dt.float32
BF16 = mybir.dt.bfloat16
AX = mybir.AxisListType
OP = mybir.AluOpType

B, C, HH, WW = 16, 512, 64, 64
N = HH * WW
IC = 256
NCORES = 8
NB = B // NCORES  # batches per core
P = 128
KC = C // P   # 4 contraction chunks over C
MI = IC // P  # 2 chunks over IC
NN = 8        # n-chunks
NT = N // NN  # 512
OUT_SIZES = [1, 3, 6, 8]
S = sum(o * o for o in OUT_SIZES)  # 110
SPP_OFF = {1: 0, 3: 1, 6: 10, 8: 46}


def _bounds(n, o):
    return [((i * n) // o, ((i + 1) * n + o - 1) // o) for i in range(o)]


def _atoms():
    bs = set()
    for o in OUT_SIZES:
        for s, e in _bounds(HH, o):
            bs.add(s); bs.add(e)
    bs = sorted(bs)
    return [(bs[i], bs[i + 1]) for i in range(len(bs) - 1)]


ATOMS = _atoms()          # 16 atomic row intervals
NA = len(ATOMS)


def _bin_atom_ranges(o):
    """For each bin of size-o pooling: (first_atom_idx, last_atom_idx_excl)."""
    out = []
    for s, e in _bounds(HH, o):
        a0 = next(i for i, (as_, _) in enumerate(ATOMS) if as_ == s)
        a1 = next(i for i, (_, ae) in enumerate(ATOMS) if ae == e) + 1
        out.append((a0, a1))
    return out


def _grouped(items):
    """Group indices j of (start, length) items into classes {j = r mod m} where
    each class has constant length and arithmetic starts. Returns list of
    (j0, m, cnt, start0, dstart, length)."""
    n = len(items)
    for m in range(1, n + 1):
        groups = []
        ok = True
        for r in range(m):
            js = list(range(r, n, m))
            lens = {items[j][1] for j in js}
            if len(lens) != 1:
                ok = False; break
            starts = [items[j][0] for j in js]
            d = starts[1] - starts[0] if len(starts) > 1 else 0
            if any(starts[i + 1] - starts[i] != d for i in range(len(starts) - 1)):
                ok = False; break
            groups.append((r, m, len(js), starts[0], d, lens.pop()))
        if ok:
            return groups
    raise AssertionError


ROW_GROUPS = {o: _grouped([(a0, a1 - a0) for a0, a1 in _bin_atom_ranges(o)])
              for o in OUT_SIZES}
COL_GROUPS = {o: _grouped([(s, e - s) for s, e in _bounds(WW, o)])
              for o in OUT_SIZES}
RB_OFF = {}  # row-bin output offset (units of 64 cols) per o
_off = 0
for _o in OUT_SIZES:
    RB_OFF[_o] = _off
    _off += _o
RB_TOT = _off  # 18


def mk(ap_base, off_elems, dims):
    """Custom free-dim AP over a tile's base AP: dims = [(step, count), ...]."""
    part = list(ap_base.ap[0])
    return AP(tensor=ap_base.tensor, offset=ap_base.offset + off_elems,
              ap=[part] + [[s, c] for s, c in dims])


def build():
    nc = bacc.Bacc("TRN2", target_bir_lowering=False, debug=False,
                   num_devices=NCORES)
    x_ext = nc.declare_dram_parameter("x", [NB, C, N], F32, isOutput=False)
    y_ext = nc.declare_dram_parameter("y", [NB, C, N], F32, isOutput=False)
    wth_ext = nc.declare_dram_parameter("wthT", [C, IC], BF16, isOutput=False)
    wg_ext = nc.declare_dram_parameter("wgT", [C, IC], BF16, isOutput=False)
    wphi_ext = nc.declare_dram_parameter("wphi", [IC, C], F32, isOutput=False)
    wmk_ext = nc.declare_dram_parameter("wmkT", [IC, C], BF16, isOutput=False)
    out_ext = nc.declare_dram_parameter("out", [NB, C, N], F32, isOutput=True)

    with tile.TileContext(nc) as tc:
        with (
            tc.tile_pool(name="w", bufs=1) as wp,
            tc.tile_pool(name="io32", bufs=6) as iop,
            tc.tile_pool(name="hlc", bufs=10) as hlp,
            tc.tile_pool(name="pool", bufs=1) as pp,
            tc.tile_pool(name="attn", bufs=2) as ap_,
            tc.tile_pool(name="ostg", bufs=6) as osp,
            tc.tile_pool(name="psum", bufs=2, space="PSUM") as ps,
        ):
            # ---- weights (resident) ----
            wth_t = [wp.tile([P, IC], BF16, tag=f"wth{k}", name=f"wth{k}") for k in range(KC)]
            wg_t = [wp.tile([P, IC], BF16, tag=f"wg{k}", name=f"wg{k}") for k in range(KC)]
            wphi_t = [wp.tile([P, C], F32, tag=f"wphi{k}", name=f"wphi{k}") for k in range(MI)]
            wmk_t = [wp.tile([P, C], BF16, tag=f"wmk{k}", name=f"wmk{k}") for k in range(MI)]
            for k in range(KC):
                nc.sync.dma_start(wth_t[k][:], wth_ext[k * P:(k + 1) * P, :])
                nc.sync.dma_start(wg_t[k][:], wg_ext[k * P:(k + 1) * P, :])
            for k in range(MI):
                nc.sync.dma_start(wphi_t[k][:], wphi_ext[k * P:(k + 1) * P, :])
                nc.sync.dma_start(wmk_t[k][:], wmk_ext[k * P:(k + 1) * P, :])

            for b in range(NB):
                emit_batch(nc, tc, b, x_ext, y_ext, out_ext,
                           wth_t, wg_t, wphi_t, wmk_t, iop, hlp, pp, ap_, osp, ps)

    nc.compile()
    return nc


def spp_reduce_from_psum(nc, pt, nn, ratoms, mi, dtag):
    """Stage R: row-atom max-pool directly from a conv psum chunk [128,512]
    (8 image rows). Writes ratom[:, a*WW : (a+1)*WW]."""
    base = pt[:]
    for ai, (s, e) in enumerate(ATOMS):
        if s >= 8 * nn and e <= 8 * (nn + 1):
            ls = s - 8 * nn
            src = mk(base, ls * WW, [(1, WW), (WW, e - s)])
            dst = mk(ratoms[mi][:], ai * WW, [(1, WW)])
            nc.vector.reduce_max(dst, src, axis=AX.X)


def spp_bins(nc, ratoms, rbs, spps):
    """Stages B1 (row bins from row atoms) + B2 (col bins, strided groups)."""
    for mi in range(MI):
        ratom, rb, spp = ratoms[mi][:], rbs[mi][:], spps[mi][:]
        for o in OUT_SIZES:
            for (r, m, cnt, a0, da, ln) in ROW_GROUPS[o]:
                # in: [P, (i cnt, stride da*WW), (w WW, 1), (atoms ln, WW)]
                src = mk(ratom, a0 * WW, [(da * WW, cnt), (1, WW), (WW, ln)])
                dst = mk(rb, (RB_OFF[o] + r) * WW, [(m * WW, cnt), (1, WW)])
                nc.vector.reduce_max(dst, src, axis=AX.X)
            for (r, m, cnt, s0, ds, ln) in COL_GROUPS[o]:
                # in: [P, (i o, stride WW), (j cnt, stride ds), (w ln, 1)]
                src = mk(rb, RB_OFF[o] * WW + s0, [(WW, o), (ds, cnt), (1, ln)])
                dst = mk(spp, SPP_OFF[o] + r, [(o, o), (m, cnt)])
                nc.vector.reduce_max(dst, src, axis=AX.X)


def emit_batch(nc, tc, b, x_ext, y_ext, out_ext, wth_t, wg_t, wphi_t, wmk_t,
               iop, hlp, pp, ap_, osp, ps):
    # ---- load y ----
    y_t = []
    for k in range(KC):
        t = iop.tile([P, N], F32, tag="io32", name=f"yt_{b}_{k}")
        nc.sync.dma_start(t[:], y_ext[b, k * P:(k + 1) * P, :])
        y_t.append(t)
    # x loaded now too (DMA engines have slack; io32 pool slots gate it)
    x_t = []
    for k in range(KC):
        t = iop.tile([P, N], F32, tag="io32", name=f"xt_{b}_{k}")
        nc.sync.dma_start(t[:], x_ext[b, k * P:(k + 1) * P, :])
        x_t.append(t)

    ratoms_th = [pp.tile([P, NA * WW], F32, tag=f"rath{mi}", name=f"rath{mi}_{b}") for mi in range(MI)]
    ratoms_g = [pp.tile([P, NA * WW], BF16, tag=f"rag{mi}", name=f"rag{mi}_{b}") for mi in range(MI)]

    # ---- conv phase: theta (split2) + g (bf16), pooling fused from psum ----
    for nn in range(NN):
        yh_c, yl_c = [], []
        for k in range(KC):
            ysl = y_t[k][:, nn * NT:(nn + 1) * NT]
            h = hlp.tile([P, NT], BF16, tag="hlc", name=f"h_{b}_{nn}_{k}")
            nc.scalar.copy(h[:], ysl)                      # ACT: hi cast
            l = hlp.tile([P, NT], BF16, tag="hlc", name=f"l_{b}_{nn}_{k}")
            nc.gpsimd.tensor_sub(l[:], ysl, h[:])          # lo residual
            yh_c.append(h); yl_c.append(l)
        for mi in range(MI):
            pt = ps.tile([P, NT], F32, tag=f"conv{mi}", name=f"pth{mi}_{b}_{nn}")
            for k in range(KC):
                nc.tensor.matmul(pt[:], wth_t[k][:, mi * P:(mi + 1) * P],
                                 yh_c[k][:], start=(k == 0), stop=False)
            for k in range(KC):
                nc.tensor.matmul(pt[:], wth_t[k][:, mi * P:(mi + 1) * P],
                                 yl_c[k][:], start=False, stop=(k == KC - 1))
            spp_reduce_from_psum(nc, pt, nn, ratoms_th, mi, "th")
        for mi in range(MI):
            pg = ps.tile([P, NT], F32, tag=f"conv{mi}", name=f"pg{mi}_{b}_{nn}")
            for k in range(KC):
                nc.tensor.matmul(pg[:], wg_t[k][:, mi * P:(mi + 1) * P],
                                 yh_c[k][:], start=(k == 0), stop=(k == KC - 1))
            spp_reduce_from_psum(nc, pg, nn, ratoms_g, mi, "g")

    # ---- SPP bins ----
    rbs_th = [pp.tile([P, RB_TOT * WW], F32, tag=f"rbth{mi}", name=f"rbth{mi}_{b}") for mi in range(MI)]
    rbs_g = [pp.tile([P, RB_TOT * WW], BF16, tag=f"rbg{mi}", name=f"rbg{mi}_{b}") for mi in range(MI)]
    spp_th = [pp.tile([P, S], F32, tag=f"spth{mi}{b % 2}", name=f"spth{mi}_{b}") for mi in range(MI)]
    spp_g = [pp.tile([P, S], BF16, tag=f"spg{mi}{b % 2}", name=f"spg{mi}_{b}") for mi in range(MI)]
    spp_bins(nc, ratoms_th, rbs_th, spp_th)
    spp_bins(nc, ratoms_g, rbs_g, spp_g)

    # ---- M1T = w_phi^T @ th_spp (fp32), evac to bf16 lhsT chunks ----
    m1_bf = []
    for mc in range(KC):
        pm = ps.tile([P, S], F32, tag="psmall", name=f"pm1_{b}_{mc}")
        for k in range(MI):
            nc.tensor.matmul(pm[:], wphi_t[k][:, mc * P:(mc + 1) * P],
                             spp_th[k][:], start=(k == 0), stop=(k == MI - 1))
        m = pp.tile([P, S], BF16, tag=f"m1_{mc}{b % 2}", name=f"m1b_{b}_{mc}")
        nc.scalar.copy(m[:], pm[:])
        m1_bf.append(m)

    # ---- scores^T = M1^T @ (xh+xl), softmax stats ----
    sc_sb = pp.tile([S, N], F32, tag="scsb")
    gm = pp.tile([S, NN + 8], F32, tag=f"gm{b % 2}")
    for nn in range(NN):
        xh_c = []
        for k in range(KC):
            xsl = x_t[k][:, nn * NT:(nn + 1) * NT]
            h = hlp.tile([P, NT], BF16, tag="hlc", name=f"xh_{b}_{nn}_{k}")
            nc.scalar.copy(h[:], xsl)
            xh_c.append(h)
        psc = ps.tile([S, NT], F32, tag="pattn", name=f"psc_{b}_{nn}")
        for k in range(KC):
            nc.tensor.matmul(psc[:], m1_bf[k][:], xh_c[k][:],
                             start=(k == 0), stop=(k == KC - 1))
        nc.vector.reduce_max(gm[:, nn:nn + 1], psc[:], axis=AX.X)
        nc.scalar.copy(sc_sb[:, nn * NT:(nn + 1) * NT], psc[:])

    gmax = pp.tile([S, 1], F32, tag=f"gmax{b % 2}")
    nc.vector.reduce_max(gmax[:], gm[:, 0:NN], axis=AX.X)
    ngmax = pp.tile([S, 1], F32, tag=f"ngmax{b % 2}")
    nc.vector.tensor_scalar_mul(ngmax[:], gmax[:], -1.0)

    e_bf = pp.tile([S, N], BF16, tag="ebf")
    dsum = pp.tile([S, 1], F32, tag=f"dsum{b % 2}")
    nc.scalar.activation(e_bf[:], sc_sb[:], mybir.ActivationFunctionType.Exp,
                         bias=ngmax[:], scale=1.0, accum_out=dsum[:])
    rden = pp.tile([S, 1], F32, tag=f"rden{b % 2}")
    nc.vector.reciprocal(rden[:], dsum[:])

    # ---- M2T = (g_spp^T @ w_mask^T) * rden, bf16 ----
    pm2 = ps.tile([S, C], F32, tag="psmall", name=f"pm2_{b}")
    for k in range(MI):
        nc.tensor.matmul(pm2[:], spp_g[k][:], wmk_t[k][:],
                         start=(k == 0), stop=(k == MI - 1))
    m2_bf = pp.tile([S, C], BF16, tag=f"m2{b % 2}")
    nc.vector.tensor_scalar_mul(m2_bf[:], pm2[:], rden[:])

    # ---- mask = M2^T @ e ; out = mask + x ----
    for mc in range(KC):
        for nn in range(NN):
            pk = ps.tile([P, NT], F32, tag="pattn", name=f"pk_{b}_{mc}_{nn}")
            nc.tensor.matmul(pk[:], m2_bf[:, mc * P:(mc + 1) * P],
                             e_bf[:, nn * NT:(nn + 1) * NT],
                             start=True, stop=True)
            xsl = x_t[mc][:, nn * NT:(nn + 1) * NT]
            if (mc + nn) % 2 == 0:
                # DVE: psum + x -> x_t in place (evac fused with residual)
                nc.vector.tensor_add(xsl, pk[:], xsl)
                nc.sync.dma_start(
                    out_ext[b, mc * P:(mc + 1) * P, nn * NT:(nn + 1) * NT], xsl)
            else:
                o = osp.tile([P, NT], F32, tag="ostg", name=f"ost_{b}_{mc}_{nn}")
                nc.scalar.copy(o[:], pk[:])
                nc.gpsimd.tensor_add(o[:], o[:], xsl)
                nc.sync.dma_start(
                    out_ext[b, mc * P:(mc + 1) * P, nn * NT:(nn + 1) * NT], o[:])


_NC_CACHE = {}


def _get_nc():
    if "nc" not in _NC_CACHE:
        _NC_CACHE["nc"] = build()
    return _NC_CACHE["nc"]


def kernel(x, y, w_phi, w_theta, w_g, w_mask):
    x = np.ascontiguousarray(np.asarray(x, dtype=np.float32))
    y = np.ascontiguousarray(np.asarray(y, dtype=np.float32))
    bf = ml_dtypes.bfloat16
    wthT = np.ascontiguousarray(np.asarray(w_theta, np.float32).T).astype(bf)
    wgT = np.ascontiguousarray(np.asarray(w_g, np.float32).T).astype(bf)
    wphi = np.ascontiguousarray(np.asarray(w_phi, np.float32))
    wmkT = np.ascontiguousarray(np.asarray(w_mask, np.float32).T).astype(bf)

    nc = _get_nc()
    in_maps = []
    for c in range(NCORES):
        sl = slice(c * NB, (c + 1) * NB)
        in_maps.append({
            "x": x[sl].reshape(NB, C, N),
            "y": y[sl].reshape(NB, C, N),
            "wthT": wthT, "wgT": wgT, "wphi": wphi, "wmkT": wmkT,
        })
    res = run_bass_kernel_spmd(nc, in_maps, core_ids=list(range(NCORES)))
    out = np.concatenate([r["out"].reshape(NB, C, HH, WW) for r in res.results],
                         axis=0)
    return out
